# revision 22
# baseline (speedup 1.0000x reference)
"""Trainium2 Bass kernel for nn_AllOutputsGRU.

Model: L=2 independent GRU layers over the SAME input x (ensemble style),
output = mean over layers of the full hidden-state sequence (T, B, H).

Sharding: 8 cores = 2 layers x 4 batch-groups (16 samples each); every core
runs a fully independent scan (no collectives). Transposed layout: gates on
partitions (3H -> 12 m-tiles of 128), batch is the moving dim.

Per-step schedule. The PE sem-increment stream (~35ns/instr) lags the MM
issue rate (~27ns), so a PSUM group's release to consumers trails its last
matmul by up to ~0.4us; the schedule is balanced around that: PSUM groups
run in R -> N -> Z order (k-outer within each group) so that the two
dependency paths
  t1 = r*gn -> sn = t1+xi_n   (gated by sigmoid(r) at R-release and N-release)
  oz = sigmoid(-z)            (gated by Z-release, ACT-ordered before tanh)
converge at tanh simultaneously. After tanh only v = oz*n and the h' writes
remain; u' = (oz-1)*h_prev = -z*h_prev (fused scalar_tensor_tensor) runs in
the tanh shadow, h' = v - u'. h' is written as two bf16 tiles (k01 first)
so the next sweep's k-outer matmuls start on the first half ~180ns early.
The next step's injects and the next chunk's input-projection matmuls
execute during the current pointwise tail; xi PSUM->SBUF copies are packed
into steps [2,50) and FIFO-pinned behind h' so the whole xi tile is ready
well before the next chunk's first injects wait on it. bf16 weights/moving
operands (FWL), fp32 PSUM accumulate, fp32 h state.
"""

import sys

import numpy as np

try:
    import concourse.bass as bass  # noqa: F401
except ImportError:
    sys.path.insert(0, "/opt/trn_rl_repo")

import concourse.bass as bass
import concourse.bacc as bacc
import concourse.mybir as mybir
import concourse.tile as tile
from concourse.tile import add_dep_helper
from concourse.bass import ds
from concourse.bass_utils import run_bass_kernel_spmd

import ml_dtypes

BF16 = ml_dtypes.bfloat16

# Problem sizes (hardcoded per task spec).
T, B, F, H, L = 1024, 64, 256, 512, 2
NCORES = 8
NBG = 4          # batch groups
Bc = B // NBG    # 16 samples per core
Tc = 64          # timesteps per chunk
NCHUNK = T // Tc         # 16
KH = H // 128            # 4  k-chunks of the recurrent contraction
KF = F // 128            # 2  k-chunks of the input contraction
MG = H // 128            # 4  m-tiles per gate
NM = 3 * MG              # 12 m-tiles total
COLS = Tc * Bc           # 1024 free columns per chunk
XT_COLS = T * Bc + 2 * COLS  # padded so prefetch of chunks 16/17 is in-bounds

FP32 = mybir.dt.float32
DBF16 = mybir.dt.bfloat16
AF = mybir.ActivationFunctionType
ALU = mybir.AluOpType


def build_nc():
    nc = bacc.Bacc("TRN2", target_bir_lowering=False, debug=False)

    xt_d = nc.declare_dram_parameter("xt", [KF, 128, XT_COLS], DBF16, isOutput=False)
    wih_d = nc.declare_dram_parameter("wih", [KF, 128, 3 * H], DBF16, isOutput=False)
    whh_d = nc.declare_dram_parameter("whh", [KH, 128, 3 * H], DBF16, isOutput=False)
    iden_d = nc.declare_dram_parameter("iden", [128, 128], DBF16, isOutput=False)
    bhnb_d = nc.declare_dram_parameter("bhnb", [128, MG, Bc], DBF16, isOutput=False)
    bias_d = nc.declare_dram_parameter("bias", [128, NM], FP32, isOutput=False)
    out_d = nc.declare_dram_parameter("out", [KH, 128, T * Bc], FP32, isOutput=True)

    with tile.TileContext(nc) as tc:
        with (
            tc.tile_pool(name="const", bufs=1) as cpool,
            tc.tile_pool(name="xt", bufs=1) as xtpool,
            tc.tile_pool(name="xi", bufs=1) as xipool,
            tc.tile_pool(name="hs", bufs=1) as hspool,
            tc.tile_pool(name="tmp", bufs=2) as tmp,
            tc.tile_pool(name="rp", bufs=1, space="PSUM") as rpool,
            tc.tile_pool(name="zp", bufs=1, space="PSUM") as zpool,
            tc.tile_pool(name="np", bufs=2, space="PSUM") as npool,
            tc.tile_pool(name="xip", bufs=3, space="PSUM") as xippool,
        ):
            whh_t = cpool.tile([128, KH, 3 * H], DBF16, tag="whh")
            wih_t = cpool.tile([128, KF, 3 * H], DBF16, tag="wih")
            iden_t = cpool.tile([128, 128], DBF16, tag="iden")
            bhnb_t = cpool.tile([128, MG, Bc], DBF16, tag="bhnb")
            bias_t = cpool.tile([128, NM], FP32, tag="bias")
            h16 = [[cpool.tile([128, KH // 2, Bc], DBF16, tag=f"h16_{p}{h}",
                                name=f"h16_{p}{h}") for h in range(2)] for p in range(2)]
            xt_t = [xtpool.tile([128, KF, COLS], DBF16, tag=f"xt_{p}", name=f"xt_{p}") for p in range(2)]
            xi_t = [[xipool.tile([128, NM, Tc // 2, Bc], DBF16, tag=f"xi_{p}{h}",
                                 name=f"xi_{p}{h}") for h in range(2)] for p in range(2)]
            hs_t = [hspool.tile([128, KH, Tc, Bc], FP32, tag=f"hs_{p}", name=f"hs_{p}") for p in range(2)]

            # Load x chunk 0 first (it gates the prologue xi projection),
            # then weights/biases (not needed until the first sweep).
            for k in range(KF):
                nc.sync.dma_start(xt_t[0][:, k, :], xt_d[k, :, 0:COLS])
            for k in range(KF):
                nc.sync.dma_start(wih_t[:, k, :], wih_d[k])
            nc.sync.dma_start(bias_t[:], bias_d[:])
            for k in range(KH):
                nc.sync.dma_start(whh_t[:, k, :], whh_d[k])
            nc.sync.dma_start(iden_t[:], iden_d[:])
            nc.sync.dma_start(bhnb_t[:, :, :], bhnb_d[:])

            # h_{-1} = 0: zero the bf16 h16[1] and the f32 slot that global
            # step 0 reads (last column of hs buffer B).
            nc.vector.memset(h16[1][0][:, :, :], 0.0)
            nc.vector.memset(h16[1][1][:, :, :], 0.0)
            nc.vector.memset(hs_t[1][:, :, Tc - 1, :], 0.0)

            NCH = COLS // 512  # column-halves per chunk (PSUM bank limit)
            TH = Tc // NCH
            # ch-major: the ch=0 units complete first, so a chunk's first
            # sweep gates on only half the units (xi is a pair of half-tiles).
            XI_UNITS = [(m, ch) for ch in range(NCH) for m in range(NM)]

            def emit_xi_mms(xt_buf, m, ch):
                """PE half of one xi unit: xp = (x_chunk @ W_ih^T)[m] for one
                column-half. Returns the PSUM tile for the deferred copy."""
                xp = xippool.tile([128, TH, Bc], FP32, tag="xp")
                for k in range(KF):
                    nc.tensor.matmul(
                        xp[:],
                        wih_t[:, k, m * 128:(m + 1) * 128],
                        xt_buf[:, k, ch * 512:(ch + 1) * 512],
                        start=(k == 0),
                        stop=(k == KF - 1),
                    )
                return xp

            def emit_xi_copy(xi_buf, m, ch, xp, copy_eng):
                if copy_eng is nc.vector:
                    return copy_eng.tensor_scalar_add(
                        xi_buf[ch][:, m, :, :], xp[:], bias_t[:, m:m + 1])
                return nc.scalar.activation(
                    xi_buf[ch][:, m, :, :], xp[:],
                    AF.Identity, bias=bias_t[:, m:m + 1], scale=1.0)

            def emit_xi(xt_buf, xi_buf):
                # Prologue only: alternate DVE/ACT so the 24 copies stream on
                # two engines instead of serializing on DVE.
                for i, (m, ch) in enumerate(XI_UNITS):
                    xp = emit_xi_mms(xt_buf, m, ch)
                    emit_xi_copy(xi_buf, m, ch, xp,
                                 nc.vector if i % 2 == 0 else nc.scalar)

            def emit_injects(s, rp, za, zb, gn, xi_buf):
                """PSUM accumulation-group openers; h-independent, so they
                run during the previous step's tail."""
                xb = xi_buf[s // TH]
                sh = s % TH
                nc.tensor.matmul(rp[:, :, :], iden_t[:],
                                 xb[:, 0:MG, sh, :], start=True, stop=False)
                nc.tensor.matmul(za[:, :, :], iden_t[:],
                                 xb[:, MG:MG + 2, sh, :], start=True, stop=False)
                nc.tensor.matmul(zb[:, :, :], iden_t[:],
                                 xb[:, MG + 2:2 * MG, sh, :], start=True, stop=False)
                nc.tensor.matmul(gn[:, :, :], iden_t[:],
                                 bhnb_t[:, :, :], start=True, stop=False)

            def emit_scan(xi_buf, hs_buf, hs_prev, xi_next=None):
                """Tc GRU steps; reads xi, writes hs_buf (f32 h history).
                xi_next = (xt_buf, xi_out): next chunk's input projection,
                interleaved so it fills idle windows. Per step, the PE stream
                is [R/Z/N injects, 48 h-MMs, xi MMs]; the injects and xi MMs
                of a step execute during the previous step's pointwise tail."""
                nxu = len(XI_UNITS) if xi_next is not None else 0
                for s in range(Tc):
                    rp = rpool.tile([128, MG, Bc], FP32, tag="rp")
                    za = zpool.tile([128, MG // 2, Bc], FP32, tag="za")
                    zb = zpool.tile([128, MG // 2, Bc], FP32, tag="zb")
                    gn = npool.tile([128, MG, Bc], FP32, tag="gn")
                    emit_injects(s, rp, za, zb, gn, xi_buf)
                    hin = h16[(s + 1) % 2]
                    # k-outer within each group: the first MG MMs need only
                    # the k01 half of h', so the sweep starts on h16a while
                    # h16b lands.
                    for ptile, mtiles in ((rp, (0, 1, 2, 3)),
                                          (gn, (8, 9, 10, 11)),
                                          (za, (4, 5)), (zb, (6, 7))):
                        nmt = len(mtiles)
                        for k in range(KH):
                            for m, mm in enumerate(mtiles):
                                nc.tensor.matmul(
                                    ptile[:, m, :],
                                    whh_t[:, k, mm * 128:(mm + 1) * 128],
                                    hin[k // 2][:, k % 2, :],
                                    start=False,
                                    stop=(k == KH - 1 and m == nmt - 1),
                                )
                    # Next chunk's xi matmuls fill the PE tail gap; their
                    # PSUM->SBUF copies are deferred below the chain so they
                    # never block it in the DVE/GPSIMD FIFOs. Units are packed
                    # into steps [2, 50) so the xi tile is complete well
                    # before the next chunk's first injects wait on it.
                    XI_S0, XI_S1 = 2, 50
                    pending_xi = []
                    if XI_S0 <= s < XI_S1:
                        u0 = (s - XI_S0) * nxu // (XI_S1 - XI_S0)
                        u1 = (s + 1 - XI_S0) * nxu // (XI_S1 - XI_S0)
                        for u in range(u0, u1):
                            m, ch = XI_UNITS[u]
                            pending_xi.append(
                                (m, ch, emit_xi_mms(xi_next[0], m, ch), nc.vector))
                    hprev = hs_prev[:, :, Tc - 1, :] if s == 0 else hs_buf[:, :, s - 1, :]
                    r = tmp.tile([128, MG, Bc], FP32, tag="r")
                    nc.scalar.activation(r[:], rp[:], AF.Sigmoid)
                    t1 = tmp.tile([128, MG, Bc], FP32, tag="t1")
                    nc.vector.tensor_mul(t1[:], r[:], gn[:])
                    sn = tmp.tile([128, MG, Bc], FP32, tag="sn")
                    sn_i = nc.vector.tensor_add(
                        sn[:], t1[:], xi_buf[s // TH][:, 2 * MG:, s % TH, :])
                    # oz = sigmoid(-z) goes BEFORE tanh on the ACT FIFO and
                    # is split to match the Z_a/Z_b PSUM groups: oz_a's
                    # isolated ACT startup runs during the sweep's release
                    # lag, and oz_b (released with the last sweep MM)
                    # executes pipelined right behind it.
                    oz = tmp.tile([128, MG, Bc], FP32, tag="oz")
                    oza_i = nc.scalar.activation(oz[:, 0:2, :], za[:],
                                                 AF.Sigmoid, scale=-1.0)
                    ozb_i = nc.scalar.activation(oz[:, 2:4, :], zb[:],
                                                 AF.Sigmoid, scale=-1.0)
                    add_dep_helper(ozb_i.ins, oza_i.ins, sync=False,
                                   reason="ACT order: oz_a before oz_b")
                    n = tmp.tile([128, MG, Bc], FP32, tag="n")
                    tanh_i = nc.scalar.activation(n[:], sn[:], AF.Tanh)
                    add_dep_helper(tanh_i.ins, ozb_i.ins, sync=False,
                                   reason="ACT order: oz before tanh")
                    up = tmp.tile([128, MG, Bc], FP32, tag="up")
                    up_i = nc.vector.scalar_tensor_tensor(
                        up[:], oz[:], 1.0, hprev, op0=ALU.subtract, op1=ALU.mult)
                    add_dep_helper(up_i.ins, sn_i.ins, sync=False,
                                   reason="DVE order: up after sn")
                    v = tmp.tile([128, MG, Bc], FP32, tag="v")
                    nc.vector.tensor_mul(v[:], oz[:], n[:])
                    # h' = v - u' : bf16 copy feeds the next matmul sweep,
                    # f32 copy (gpsimd) is the carried state / output.
                    h16_i = nc.vector.tensor_sub(h16[s % 2][0][:, :, :],
                                                 v[:, 0:2, :], up[:, 0:2, :])
                    h16b_i = nc.vector.tensor_sub(h16[s % 2][1][:, :, :],
                                                  v[:, 2:4, :], up[:, 2:4, :])
                    add_dep_helper(h16b_i.ins, h16_i.ins, sync=False,
                                   reason="DVE order: h' k01 before k23")
                    nc.gpsimd.tensor_sub(hs_buf[:, :, s, :], v[:], up[:])
                    for m, ch, xp, eng in pending_xi:
                        cp_i = emit_xi_copy(xi_next[1], m, ch, xp, eng)
                        add_dep_helper(cp_i.ins, h16_i.ins, sync=False,
                                       reason="DVE order: xi copy after h'")

            # Prologue: xi(0) -> xiA (x(0) DMA already issued above).
            emit_xi(xt_t[0], xi_t[0])
            for k in range(KF):
                nc.sync.dma_start(xt_t[1][:, k, :], xt_d[k, :, COLS:2 * COLS])

            HINTS = (mybir.EngineType.PE, mybir.EngineType.DVE,
                     mybir.EngineType.Activation, mybir.EngineType.Pool)
            # UNROLL=16 / single iteration: no hardware-loop wraps at all
            # (each wrap costs ~13us in loop-boundary sem resets / drain).
            UNROLL = 16
            NITER = NCHUNK // UNROLL

            def segment(i, j):
                """Scan chunk c = UNROLL*i + j (buffers c%2), produce xi for
                chunk c+1 (other buffers, interleaved), store hs, prefetch x
                for c+2."""
                p = j % 2
                q = 1 - p
                emit_scan(xi_t[p], hs_t[p], hs_t[q], xi_next=(xt_t[q], xi_t[q]))
                for hc in range(KH):
                    nc.sync.dma_start(
                        out_d[hc, :, ds(i * (UNROLL * COLS) + j * COLS, COLS)],
                        hs_t[p][:, hc, :, :],
                    )
                for k in range(KF):
                    nc.sync.dma_start(
                        xt_t[p][:, k, :],
                        xt_d[k, :, ds(i * (UNROLL * COLS) + (j + 2) * COLS, COLS)],
                    )

            with tc.For_i(0, NITER, 1, hint_engines=HINTS) as i:
                for j in range(UNROLL):
                    segment(i, j)

    nc.compile()
    return nc


_NC_CACHE = None


def _get_nc():
    global _NC_CACHE
    if _NC_CACHE is None:
        _NC_CACHE = build_nc()
    return _NC_CACHE


def _prep_core_inputs(x, W_ih, W_hh, b_ih, b_hh, layer, bg):
    xs = x[:, bg * Bc:(bg + 1) * Bc, :]                   # (T, Bc, F)
    xt = np.ascontiguousarray(np.transpose(xs, (2, 0, 1)))  # (F, T, Bc)
    xt = xt.reshape(KF, 128, T * Bc)
    xt_p = np.zeros((KF, 128, XT_COLS), np.float32)
    xt_p[:, :, :T * Bc] = xt

    wih = np.ascontiguousarray(W_ih[layer].T).reshape(KF, 128, 3 * H)
    whh = np.ascontiguousarray(W_hh[layer].T).reshape(KH, 128, 3 * H)

    bias_full = b_ih[layer].copy()
    bias_full[:2 * H] += b_hh[layer][:2 * H]
    bias = np.ascontiguousarray(bias_full.reshape(NM, 128).T)

    bhn = b_hh[layer][2 * H:].reshape(MG, 128).T          # (128, MG)
    bhnb = np.ascontiguousarray(
        np.broadcast_to(bhn[:, :, None], (128, MG, Bc)))

    return {
        "xt": xt_p.astype(BF16),
        "wih": wih.astype(BF16),
        "whh": whh.astype(BF16),
        "iden": np.eye(128, dtype=np.float32).astype(BF16),
        "bhnb": bhnb.astype(BF16),
        "bias": bias.astype(np.float32),
    }


def run_cores(x, W_ih, W_hh, b_ih, b_hh, trace=False, nc=None):
    if nc is None:
        nc = _get_nc()
    in_maps = [
        _prep_core_inputs(x, W_ih, W_hh, b_ih, b_hh, core // NBG, core % NBG)
        for core in range(NCORES)
    ]
    return run_bass_kernel_spmd(nc, in_maps, core_ids=list(range(NCORES)), trace=trace)


def assemble(results):
    out = np.zeros((T, B, H), np.float32)
    for bg in range(NBG):
        acc = None
        for layer in range(L):
            o = np.asarray(results[layer * NBG + bg]["out"], np.float32)
            hs = o.reshape(KH, 128, T, Bc).transpose(2, 3, 0, 1).reshape(T, Bc, H)
            acc = hs if acc is None else acc + hs
        out[:, bg * Bc:(bg + 1) * Bc, :] = acc / L
    return out


def kernel(x, W_ih, W_hh, b_ih, b_hh):
    x = np.asarray(x, np.float32)
    W_ih = np.asarray(W_ih, np.float32)
    W_hh = np.asarray(W_hh, np.float32)
    b_ih = np.asarray(b_ih, np.float32)
    b_hh = np.asarray(b_hh, np.float32)
    res = run_cores(x, W_ih, W_hh, b_ih, b_hh, trace=False)
    return assemble(res.results)
